# revision 1
# baseline (speedup 1.0000x reference)
"""Causal self-attention with RoPE (B=2, T=2048, D=1024, H=16, Hd=64),
sharded over 8 Trainium2 NeuronCores.

Sharding: core c -> (batch b = c // 4, head-group g = c % 4). Each core
computes the QKV projection for its 4 heads, causal attention, and a
partial output projection [T, D]; the host sums the 4 partials per batch.

Per-core layout choices (no on-device transposes needed anywhere):
  - Q, K are produced transposed [hd, t] (lhsT = Wqkv^T blocks, rhs = x^T),
    RoPE applied in that layout (half-rotation = partition swap via DMA).
  - V is produced natural [t, hd] with a ones-column per head appended so
    the PV matmul also accumulates the softmax denominator for free.
  - Scores are computed as S^T [k, q] blocks; exp runs on ScalarE straight
    out of PSUM; causal staircase masks multiply on VectorE; the PV matmul
    accumulates unnormalized O^T [65, q] (row 64 = denominator).
  - O^T is normalized per finished 512-col slice (VectorE reciprocal +
    GpSimd partition-broadcast) and feeds the output projection directly
    as lhsT; projection t-blocks are drip-fed into later attention loops.
Matmul operands are stored fp16 (PSUM accumulation is fp32); measured
output relative error vs the fp32 reference is ~7e-4.
"""
import numpy as np

B, T, D = 2, 2048, 1024
H, HD = 16, 64
N_CORES = 8
GROUPS = 4              # head groups (tensor parallel dim)
HPC = H // GROUPS       # heads per core
O_PC = HPC * HD         # 256 output features per core
NT = T // 128           # 16 key/time blocks
ND = D // 128           # 8 contraction blocks
ROPE_BASE = 10000.0

_CACHE: dict = {}

# matmul-operand storage dtype: "float16" or "bfloat16".  fp16 keeps a
# 10-bit mantissa (~8x finer than bf16); all stored values here fit fp16
# range (scores are ~N(0,1); exp(max causal score) << 65504).
MM_DT_NAME = "float16"


def _host_consts():
    if "consts" in _CACHE:
        return _CACHE["consts"]
    inv = 1.0 / (ROPE_BASE ** (np.arange(0, HD, 2, dtype=np.float32) / HD))
    t = np.arange(T, dtype=np.float32)
    fr = np.outer(t, inv)                       # [T, 32]
    c = np.cos(fr).T.astype(np.float32)         # [32, T]
    s = np.sin(fr).T.astype(np.float32)
    cosT = np.ascontiguousarray(np.tile(c, (4, 1)))   # [128, T]
    sinT = np.tile(s, (4, 1))
    for blk in range(4):
        if blk % 2 == 0:
            sinT[32 * blk:32 * blk + 32] *= -1.0
    # pre-swapped (halves exchanged within each 64-row head): multiplying by
    # this and THEN swapping partitions equals swap(ps) * sinT_signed
    perm = np.concatenate([np.arange(32, 64), np.arange(0, 32),
                           np.arange(96, 128), np.arange(64, 96)])
    sinT = np.ascontiguousarray(sinT[perm])
    m = np.zeros((128, 4 * 512), np.float32)
    p = np.arange(128)[:, None]
    cc = np.arange(512)[None, :]
    for r in range(4):
        m[:, 512 * r:512 * r + 512] = (cc >= 128 * r + p).astype(np.float32)
    _CACHE["consts"] = (cosT, sinT, m)
    return _CACHE["consts"]


def _build_nc():
    import concourse.mybir as mybir
    import concourse.tile as tile
    from concourse.bacc import Bacc

    F32 = mybir.dt.float32
    MM = getattr(mybir.dt, MM_DT_NAME)
    AF = mybir.ActivationFunctionType
    OP = mybir.AluOpType

    # Bacc (not raw Bass): its compile() spills excess semaphore waits onto
    # EventSemaphore instructions (engine instructions hold only 1 wait) and
    # inserts ACT table loads for the exp activation.
    nc = Bacc(None, target_bir_lowering=False)
    # packed host layouts (one contiguous DMA each; see _make_in_maps):
    #   xTr[tci, p, 512*db+c] = x^T[128*db+p, 512*tci+c]
    #   w*r[p, 256*db+o]     = w*T[128*db+p, o]
    #   wor[p, 1024*j+d]     = woT[128*j+p, d]
    xTr = nc.dram_tensor("xTr", [4, 128, 8 * 512], MM, kind="ExternalInput")
    wqr = nc.dram_tensor("wqr", [128, ND * O_PC], MM, kind="ExternalInput")
    wkr = nc.dram_tensor("wkr", [128, ND * O_PC], MM, kind="ExternalInput")
    wvr = nc.dram_tensor("wvr", [128, ND * O_PC], MM, kind="ExternalInput")
    wor = nc.dram_tensor("wor", [128, 2 * D], MM, kind="ExternalInput")
    cosT = nc.dram_tensor("cosT", [128, T], F32, kind="ExternalInput")
    sinT = nc.dram_tensor("sinT", [128, T], F32, kind="ExternalInput")
    mst = nc.dram_tensor("mst", [128, 2048], MM, kind="ExternalInput")
    ys = [nc.dram_tensor(f"y{p}", [T, D], F32, kind="ExternalOutput")
          for p in range(2)]

    with tile.TileContext(nc) as tc:
        with (
            tc.tile_pool(name="consts", bufs=1) as cp,
            tc.tile_pool(name="persist", bufs=1) as pp,
        ):
            # load order = consumption order: q/k weights first (gate the
            # first matmuls), then rope tables, then v/mask/out-proj weights
            wq_sb = cp.tile([128, ND * O_PC], MM, name="wq_sb")
            nc.sync.dma_start(out=wq_sb[:], in_=wqr[:, :])
            xt0 = cp.tile([128, 8 * 512], MM, name="xt0")
            nc.sync.dma_start(out=xt0[:, 0:4 * 512], in_=xTr[0, :, 0:4 * 512])
            nc.sync.dma_start(out=xt0[:, 4 * 512:], in_=xTr[0, :, 4 * 512:])
            wk_sb = cp.tile([128, ND * O_PC], MM, name="wk_sb")
            nc.sync.dma_start(out=wk_sb[:], in_=wkr[:, :])
            cos_sb = cp.tile([128, T], F32, name="cos_sb")
            nc.sync.dma_start(out=cos_sb[:], in_=cosT[:, :])
            sin_sb = cp.tile([128, T], F32, name="sin_sb")
            nc.sync.dma_start(out=sin_sb[:], in_=sinT[:, :])
            wv_sb = cp.tile([128, ND * O_PC], MM, name="wv_sb")
            nc.sync.dma_start(out=wv_sb[:], in_=wvr[:, :])
            msk_sb = cp.tile([128, 2048], MM, name="msk_sb")
            nc.sync.dma_start(out=msk_sb[:], in_=mst[:, :])
            wo_sb = cp.tile([128, 2 * D], MM, name="wo_sb")
            nc.sync.dma_start(out=wo_sb[:], in_=wor[:, :])

            # persistent activations
            qt = [pp.tile([128, T], MM, name=f"qt{p}") for p in range(2)]
            kt = [pp.tile([128, T], MM, name=f"kt{p}") for p in range(2)]
            vsb = [pp.tile([128, NT * 130], MM, name=f"vsb{p}") for p in range(2)]
            ot = [pp.tile([128, T], MM, name=f"ot{p}") for p in range(2)]
            for p in range(2):
                v4 = vsb[p].rearrange("p (t j c) -> p t j c", j=2, c=65)
                nc.vector.memset(v4[:, :, :, 64:65], 1.0)

            # Staged schedule (PSUM is the scarce resource, 8 banks):
            #   A: QKV for t-chunks 0,1            [qk psums + v psums]
            #   B: attention(pair0, qh0), interleaved with QKV chunks 2,3
            #   C: attention(pair0, qh1) + proj(p0, tb 0..7)
            #   D: attention(pair1, qh0) + proj(p0, tb 8..15)
            #   E: attention(pair1, qh1) + proj(p1, tb 0..7)
            #   F: proj(p1, tb 8..15) tail
            # Attention per (pair, kb): exp narrowed to the causally-valid
            # tail, dead 128-col strips zeroed on GpSimd, triangle mask only
            # on the diagonal 128x128 block.  Projection psums share the
            # score-psum slots; interleaved work fills PE slack while ACT
            # streams exps.
            with (
                tc.tile_pool(name="xp", bufs=3) as xp,
                tc.tile_pool(name="ropep", bufs=3) as rp,
                tc.tile_pool(name="exps", bufs=6) as ep,
                tc.tile_pool(name="nrm", bufs=3) as rnp,
                tc.tile_pool(name="prjsb", bufs=6) as jsb,
            ):
                def qk_unit(pool, bufs, tci, xt_, wsb, dst, p):
                    tsl = slice(512 * tci, 512 * tci + 512)
                    ps = pool.tile([128, 512], F32, name="psqk", tag="psqk",
                                   bufs=bufs)
                    for db in range(ND):
                        nc.tensor.matmul(
                            ps[:],
                            lhsT=wsb[:, 256 * db + 128 * p:
                                     256 * db + 128 * p + 128],
                            rhs=xt_[:, 512 * db:512 * db + 512],
                            start=(db == 0), stop=(db == ND - 1),
                        )
                    # RoPE: dst = ps*cos + swap(ps)*sin_signed.  sin_sb is
                    # pre-swapped host-side so the partition swap happens on
                    # the product, via 4 SBUF->SBUF DMAs on the GpSimd queue.
                    prod = rp.tile([128, 512], MM, name="prod", tag="prod")
                    nc.vector.tensor_tensor(prod[:], ps[:], sin_sb[:, tsl], OP.mult)
                    swp = rp.tile([128, 512], MM, name="swp", tag="swp")
                    for h2 in range(4):
                        b0 = 32 * h2
                        s0 = b0 + (32 if h2 % 2 == 0 else -32)
                        nc.gpsimd.dma_start(out=swp[b0:b0 + 32, :],
                                            in_=prod[s0:s0 + 32, :])
                    nc.vector.tensor_tensor(dst[p][:, tsl], ps[:], cos_sb[:, tsl], OP.mult)
                    nc.vector.tensor_tensor(dst[p][:, tsl], dst[p][:, tsl], swp[:], OP.add)

                def v_unit(pool, bufs, tci, xt_, tb4):
                    tb = 4 * tci + tb4
                    pv = pool.tile([128, O_PC], F32, name="psv", tag="psv",
                                   bufs=bufs)
                    for db in range(ND):
                        nc.tensor.matmul(
                            pv[:],
                            lhsT=xt_[:, 512 * db + 128 * tb4:
                                     512 * db + 128 * tb4 + 128],
                            rhs=wv_sb[:, 256 * db:256 * db + 256],
                            start=(db == 0), stop=(db == ND - 1),
                        )
                    for p in range(2):
                        dstv = vsb[p].rearrange(
                            "p (t j c) -> p t j c", j=2, c=65)[:, tb, :, 0:64]
                        srcv = pv[:, 128 * p:128 * p + 128].rearrange(
                            "p (j c) -> p j c", j=2)
                        nc.vector.tensor_copy(dstv, srcv)

                def attn_kb(pool, pvp, p, qh, kb, wide):
                    """One key-block for BOTH heads of the pair: the two
                    heads' S^T matmuls (contract=64, array rows 0-63 vs
                    64-127 via base-partition tile_position) are emitted
                    back-to-back so they overlap in the PE array; then both
                    exps, then both PV accumulations.  wide=True uses a
                    [128,1024] score psum + one exp per head; wide=False uses
                    per-512-slice psums (1 bank each, for bank-scarce stages)."""
                    sq_min = kb // 4
                    smin = max(0, sq_min - 2 * qh)
                    diag = sq_min >= 2 * qh
                    r = kb % 4
                    col0 = 512 * smin + 128 * r if diag else 0

                    def s_matmul(dst_ap, i, s):
                        nc.tensor.matmul(
                            dst_ap,
                            lhsT=kt[p][64 * i:64 * i + 64,
                                       128 * kb:128 * kb + 128],
                            rhs=qt[p][64 * i:64 * i + 64,
                                      1024 * qh + 512 * s:
                                      1024 * qh + 512 * s + 512],
                            start=True, stop=True,
                        )

                    ess = []
                    if wide:
                        spss = []
                        for i in range(2):
                            sps = pool.tile([128, 1024], F32, name="sps",
                                            tag="sps", bufs=2)
                            for s in range(smin, 2):
                                s_matmul(sps[:, 512 * s:512 * s + 512], i, s)
                            spss.append(sps)
                        for i in range(2):
                            es = ep.tile([128, 1024], MM, name="es", tag="es")
                            if diag and r > 0:
                                nc.gpsimd.memset(es[:, 512 * smin:col0], 0.0)
                            nc.scalar.activation(es[:, col0:],
                                                 spss[i][:, col0:], AF.Exp)
                            ess.append(es)
                    else:
                        sp5s = []
                        for i in range(2):
                            for s in range(smin, 2):
                                sp5 = pool.tile([128, 512], F32, name="sp5",
                                                tag="sps", bufs=2)
                                s_matmul(sp5[:], i, s)
                                sp5s.append((i, s, sp5))
                        for i in range(2):
                            es = ep.tile([128, 1024], MM, name="es", tag="es")
                            if diag and r > 0:
                                nc.gpsimd.memset(es[:, 512 * smin:col0], 0.0)
                            ess.append(es)
                        for i, s, sp5 in sp5s:
                            c0s = col0 - 512 * s if s == smin and diag else 0
                            nc.scalar.activation(
                                ess[i][:, 512 * s + c0s:512 * s + 512],
                                sp5[:, c0s:], AF.Exp)
                    for i in range(2):
                        if diag:
                            nc.vector.tensor_tensor(
                                ess[i][:, col0:col0 + 128],
                                ess[i][:, col0:col0 + 128],
                                msk_sb[:, 0:128], OP.mult)
                    for i in range(2):
                        for s in range(smin, 2):
                            sq = 2 * qh + s
                            nc.tensor.matmul(
                                pvp[i][:, 512 * s:512 * s + 512],
                                lhsT=vsb[p][:, 130 * kb + 65 * i:
                                            130 * kb + 65 * i + 65],
                                rhs=ess[i][:, 512 * s:512 * s + 512],
                                start=(kb == 0), stop=(kb == 4 * sq + 3),
                            )

                ready_proj = []

                def norm_slice(pvp, p, qh, sl):
                    """Normalize one finished 512-col slice of O^T and mark
                    its 4 projection t-blocks ready."""
                    sq = 2 * qh + sl
                    for i in range(2):
                        rec = rnp.tile([1, 512], F32, name="rec", tag="rec")
                        nc.vector.reciprocal(
                            rec[:], pvp[i][64:65, 512 * sl:512 * sl + 512])
                        bc = rnp.tile([64, 512], F32, name="bc", tag="bc")
                        nc.gpsimd.partition_broadcast(bc[:], rec[0:1, :])
                        nc.vector.tensor_tensor(
                            ot[p][64 * i:64 * i + 64, 512 * sq:512 * sq + 512],
                            pvp[i][0:64, 512 * sl:512 * sl + 512],
                            bc[:], OP.mult)
                    ready_proj.extend((p, 4 * sq + j) for j in range(4))

                def maybe_norm(pvp, p, qh, kb):
                    # slice sq finishes accumulating at kb == 4*sq + 3
                    sq = (kb - 3) // 4
                    if kb >= 3 and (kb - 3) % 4 == 0 and sq >= 2 * qh:
                        norm_slice(pvp, p, qh, sq - 2 * qh)

                def make_pvp(pool):
                    pvp = []
                    for i in range(2):
                        t_ = pool.tile([65, 1024], F32, name=f"pvp{i}",
                                       tag=f"pv{i}", bufs=1)
                        pvp.append(t_)
                    return pvp

                def emit_proj_tb(pool, p, tb):
                    psy = pool.tile([128, 1024], F32, name="psy", tag="sps",
                                    bufs=2)
                    for dc in range(2):
                        nc.tensor.matmul(
                            psy[:, 512 * dc:512 * dc + 512],
                            lhsT=ot[p][:, 128 * tb:128 * tb + 128],
                            rhs=wo_sb[:, 1024 * p + 512 * dc:
                                      1024 * p + 512 * dc + 512],
                            start=True, stop=True,
                        )
                    stg = jsb.tile([128, 1024], F32, name="stg", tag="stg")
                    nc.vector.tensor_copy(stg[:], psy[:])
                    nc.sync.dma_start(
                        out=ys[p][128 * tb:128 * tb + 128, :], in_=stg[:])

                def pop_proj(pool):
                    if ready_proj:
                        emit_proj_tb(pool, *ready_proj.pop(0))

                # x chunk loads (chunk 0 was issued with the consts)
                xts = [xt0]
                for tci in range(1, 4):
                    xt_ = xp.tile([128, 8 * 512], MM, name="xt", tag="xt")
                    nc.sync.dma_start(out=xt_[:], in_=xTr[tci])
                    xts.append(xt_)

                # ---- stage A: QKV for t-chunks 0,1 ----
                with tc.tile_pool(name="psA", bufs=1, space="PSUM") as pqA:
                    for tci in range(2):
                        for wsb, dst in ((wq_sb, qt), (wk_sb, kt)):
                            for p in range(2):
                                qk_unit(pqA, 4, tci, xts[tci], wsb, dst, p)
                        for tb4 in range(4):
                            v_unit(pqA, 2, tci, xts[tci], tb4)

                # ---- stage B: attention(p0, qh0) interleaved with QKV 2,3 ----
                qkv_units = []
                for tci in range(2, 4):
                    for wsb, dst in ((wq_sb, qt), (wk_sb, kt)):
                        for p in range(2):
                            qkv_units.append(("qk", tci, wsb, dst, p))
                    for tb4 in range(4):
                        qkv_units.append(("v", tci, tb4))
                with tc.tile_pool(name="psB", bufs=1, space="PSUM") as pqB:
                    with tc.tile_pool(name="psBq", bufs=1, space="PSUM") as pqBq:
                        pvp0 = make_pvp(pqB)
                        u = 0
                        for kb in range(8):
                            attn_kb(pqB, pvp0, 0, 0, kb, wide=False)
                            for _ in range(2):
                                if u < len(qkv_units):
                                    unit = qkv_units[u]
                                    u += 1
                                    if unit[0] == "qk":
                                        _, tci, wsb, dst, p = unit
                                        qk_unit(pqBq, 1, tci, xts[tci], wsb, dst, p)
                                    else:
                                        _, tci, tb4 = unit
                                        v_unit(pqBq, 1, tci, xts[tci], tb4)
                            maybe_norm(pvp0, 0, 0, kb)
                        assert u == len(qkv_units)

                # ---- stages C/D/E + tail ----
                with tc.tile_pool(name="psC", bufs=1, space="PSUM") as pc:
                    for p, qh in ((0, 1), (1, 0), (1, 1)):
                        pvp = make_pvp(pc)
                        for kb in range(8 * qh + 8):
                            attn_kb(pc, pvp, p, qh, kb, wide=True)
                            # pop BEFORE maybe_norm: freshly-normed t-blocks
                            # wait one iteration so the PE never stalls on the
                            # normalization chain
                            if kb % 2 == 1 or len(ready_proj) > 4:
                                pop_proj(pc)
                            maybe_norm(pvp, p, qh, kb)
                    while ready_proj:
                        pop_proj(pc)
    nc.compile()
    return nc


def _get_nc():
    if "nc" not in _CACHE:
        _CACHE["nc"] = _build_nc()
    return _CACHE["nc"]


def _np_mm_dtype():
    if MM_DT_NAME == "float16":
        return np.float16
    import ml_dtypes
    return ml_dtypes.bfloat16


def _make_in_maps(x, Wqkv, Wout):
    cosT, sinT, mst = _host_consts()
    ndt = _np_mm_dtype()
    x = np.asarray(x, dtype=np.float32)
    Wqkv = np.asarray(Wqkv, dtype=np.float32)
    Wout = np.asarray(Wout, dtype=np.float32)
    in_maps = []
    for c in range(N_CORES):
        b, g = divmod(c, GROUPS)
        o0 = O_PC * g
        wq = (Wqkv[o0:o0 + O_PC] * np.float32(HD ** -0.5)).astype(np.float32)
        wk = Wqkv[D + o0:D + o0 + O_PC]
        wv = Wqkv[2 * D + o0:2 * D + o0 + O_PC]
        def pack_w(wT):
            # wT: [D, O_PC] -> [128, ND*O_PC] with d-blocks side by side
            return np.ascontiguousarray(
                wT.reshape(ND, 128, O_PC).transpose(1, 0, 2).reshape(128, -1)
            ).astype(ndt)

        xt = x[b].T  # [D, T]
        xtr = np.ascontiguousarray(
            xt.reshape(ND, 128, 4, 512).transpose(2, 1, 0, 3).reshape(4, 128, -1)
        ).astype(ndt)
        wot = Wout[:, o0:o0 + O_PC].T  # [O_PC, D]
        wor = np.ascontiguousarray(
            wot.reshape(2, 128, D).transpose(1, 0, 2).reshape(128, -1)
        ).astype(ndt)
        in_maps.append({
            "xTr": xtr,
            "wqr": pack_w(wq.T), "wkr": pack_w(wk.T), "wvr": pack_w(wv.T),
            "wor": wor,
            "cosT": cosT, "sinT": sinT, "mst": mst.astype(ndt),
        })
    return in_maps


def run(x, Wqkv, Wout, trace=False, **spmd_kwargs):
    from concourse.bass_utils import run_bass_kernel_spmd
    nc = _get_nc()
    in_maps = _make_in_maps(x, Wqkv, Wout)
    res = run_bass_kernel_spmd(nc, in_maps, core_ids=list(range(N_CORES)),
                               trace=trace, **spmd_kwargs)
    out = np.zeros((B, T, D), np.float32)
    for c in range(N_CORES):
        out[c // GROUPS] += res.results[c]["y0"]
        out[c // GROUPS] += res.results[c]["y1"]
    return out, res


def kernel(x, Wqkv, Wout, mask=None, **_ignored):
    out, _ = run(x, Wqkv, Wout, trace=False)
    return out



# revision 32
# speedup vs baseline: 1.3286x; 1.3286x over previous
"""Causal self-attention with RoPE (B=2, T=2048, D=1024, H=16, Hd=64),
sharded over 8 Trainium2 NeuronCores.

Sharding: core c -> (batch b = c // 4, head-group g = c % 4). Each core
computes the QKV projection for its 4 heads, causal attention, and a
partial output projection [T, D]; the host sums the 4 partials per batch.

v2 design (vs the O^T-form baseline): attention is organized in 512-col
q-windows processed as (w, p) pairs.  Scores are still computed as S^T
[k, q] slabs (both heads in one 2-bank psum, one merged exp on ScalarE,
diagonal slabs narrowed to the causally-valid tail), but PV runs in
NATURAL form: out[q, hd] = es_block^T @ V with a ones-column giving the
softmax denominator as an extra output column.  That cuts PV matmul
columns from 512 per (kb, head, slice) to 65 per (kb, head, q-block).
Each finished q-block is normalized with a per-partition reciprocal
(cheap - denominator is a column now), transposed back to O^T via a
128x128 identity matmul on the PE (53ns), and fed to the output
projection, which accumulates BOTH head-pairs into one psum so a single
fp16 y [T, D] leaves the chip (half the store traffic).  PV psum regions
share banks via explicit memset + start=False accumulation.  Constant
loads are spread across the SP/Act/Pool/DVE DMA queues; QKV chunks,
O-transposes and projection blocks are drip-fed into the attention kb
loops to keep the PE busy.
Matmul operands are stored fp16 (PSUM accumulation is fp32).
"""
import numpy as np

B, T, D = 2, 2048, 1024
H, HD = 16, 64
N_CORES = 8
GROUPS = 4              # head groups (tensor parallel dim)
HPC = H // GROUPS       # heads per core
O_PC = HPC * HD         # 256 output features per core
NT = T // 128           # 16 key/time blocks
ND = D // 128           # 8 contraction blocks
NW = 4                  # 512-col q windows per core
ROPE_BASE = 10000.0

_CACHE: dict = {}

MM_DT_NAME = "float16"


def _host_consts():
    if "consts" in _CACHE:
        return _CACHE["consts"]
    inv = 1.0 / (ROPE_BASE ** (np.arange(0, HD, 2, dtype=np.float32) / HD))
    t = np.arange(T, dtype=np.float32)
    fr = np.outer(t, inv)                       # [T, 32]
    c = np.cos(fr).T.astype(np.float32)         # [32, T]
    s = np.sin(fr).T.astype(np.float32)
    cosT = np.ascontiguousarray(np.tile(c, (4, 1)))   # [128, T]
    sinT = np.tile(s, (4, 1))
    for blk in range(4):
        if blk % 2 == 0:
            sinT[32 * blk:32 * blk + 32] *= -1.0
    # pre-swapped (halves exchanged within each 64-row head): multiplying by
    # this and THEN swapping partitions equals swap(ps) * sinT_signed
    perm = np.concatenate([np.arange(32, 64), np.arange(0, 32),
                           np.arange(96, 128), np.arange(64, 96)])
    sinT = np.ascontiguousarray(sinT[perm])
    # 128x128 causal staircase for the diagonal block, duplicated for the
    # two heads of a pair so one [128, 2, 128] TensorTensor masks both.
    p = np.arange(128)[:, None]
    cc = np.arange(128)[None, :]
    blkm = (cc >= p).astype(np.float32)          # [128, 128]
    mst = np.tile(blkm, (1, 2))                  # [128, 256]
    idn = np.eye(128, dtype=np.float32)          # PE-transpose identity
    _CACHE["consts"] = (cosT, sinT, mst, idn)
    return _CACHE["consts"]


def _build_nc():
    import concourse.mybir as mybir
    import concourse.tile as tile
    from concourse.bacc import Bacc

    F32 = mybir.dt.float32
    MM = getattr(mybir.dt, MM_DT_NAME)
    AF = mybir.ActivationFunctionType
    OP = mybir.AluOpType

    nc = Bacc(None, target_bir_lowering=False)
    # packed host layouts (one contiguous DMA each; see _make_in_maps):
    #   xTr[tci, p, 512*db+c] = x^T[128*db+p, 512*tci+c]
    #   w*r[p, 256*db+o]     = w*T[128*db+p, o]
    #   wor[p, 1024*j+d]     = woT[128*j+p, d]
    xTr = nc.dram_tensor("xTr", [4, 128, 8 * 512], MM, kind="ExternalInput")
    wqr = nc.dram_tensor("wqr", [128, ND * O_PC], MM, kind="ExternalInput")
    wkr = nc.dram_tensor("wkr", [128, ND * O_PC], MM, kind="ExternalInput")
    wvr = nc.dram_tensor("wvr", [128, ND * O_PC], MM, kind="ExternalInput")
    wor = nc.dram_tensor("wor", [128, 2 * D], MM, kind="ExternalInput")
    cosT = nc.dram_tensor("cosT", [128, T], MM, kind="ExternalInput")
    sinT = nc.dram_tensor("sinT", [128, T], MM, kind="ExternalInput")
    mst = nc.dram_tensor("mst", [128, 256], MM, kind="ExternalInput")
    idn = nc.dram_tensor("idn", [128, 128], MM, kind="ExternalInput")
    y = nc.dram_tensor("y", [T, D], MM, kind="ExternalOutput")

    with tile.TileContext(nc) as tc:
        with (
            tc.tile_pool(name="consts", bufs=1) as cp,
            tc.tile_pool(name="persist", bufs=1) as pp,
        ):
            # const loads spread across 3 DMA queues, consumption order;
            # wq/xt0 halves interleaved so the first matmuls start ~2.6us in
            wq_sb = cp.tile([128, ND * O_PC], MM, name="wq_sb")
            xt0 = cp.tile([128, 8 * 512], MM, name="xt0")
            nc.sync.dma_start(out=wq_sb[:, 0:4 * O_PC], in_=wqr[:, 0:4 * O_PC])
            nc.sync.dma_start(out=xt0[:, 0:4 * 512], in_=xTr[0, :, 0:4 * 512])
            nc.sync.dma_start(out=wq_sb[:, 4 * O_PC:], in_=wqr[:, 4 * O_PC:])
            nc.sync.dma_start(out=xt0[:, 4 * 512:], in_=xTr[0, :, 4 * 512:])
            wk_sb = cp.tile([128, ND * O_PC], MM, name="wk_sb")
            nc.gpsimd.dma_start(out=wk_sb[:], in_=wkr[:, :])

            cos_sb = cp.tile([128, T], MM, name="cos_sb")
            nc.scalar.dma_start(out=cos_sb[:], in_=cosT[:, :])
            sin_sb = cp.tile([128, T], MM, name="sin_sb")
            nc.scalar.dma_start(out=sin_sb[:], in_=sinT[:, :])
            xt1 = cp.tile([128, 8 * 512], MM, name="xt1")
            nc.sync.dma_start(out=xt1[:], in_=xTr[1])

            wv_sb = cp.tile([128, ND * O_PC], MM, name="wv_sb")
            nc.gpsimd.dma_start(out=wv_sb[:], in_=wvr[:, :])
            msk_sb = cp.tile([128, 256], MM, name="msk_sb")
            nc.gpsimd.dma_start(out=msk_sb[:], in_=mst[:, :])
            idn_sb = cp.tile([128, 128], MM, name="idn_sb")
            nc.gpsimd.dma_start(out=idn_sb[:], in_=idn[:, :])

            wo_sb = cp.tile([128, 2 * D], MM, name="wo_sb")
            nc.scalar.dma_start(out=wo_sb[:], in_=wor[:, :])
            xt2 = cp.tile([128, 8 * 512], MM, name="xt2")
            nc.sync.dma_start(out=xt2[:], in_=xTr[2])
            xt3 = cp.tile([128, 8 * 512], MM, name="xt3")
            nc.sync.dma_start(out=xt3[:], in_=xTr[3])

            # persistent activations
            qt = [pp.tile([128, T], MM, name=f"qt{p}") for p in range(2)]
            kt = [pp.tile([128, T], MM, name=f"kt{p}") for p in range(2)]
            vsb = [pp.tile([128, NT * 130], MM, name=f"vsb{p}") for p in range(2)]
            ot = [pp.tile([128, T], MM, name=f"ot{p}") for p in range(2)]
            for p in range(2):
                v4 = vsb[p].rearrange("p (t j c) -> p t j c", j=2, c=65)
                nc.vector.memset(v4[:, :, :, 64:65], 1.0)

            msk3 = msk_sb.rearrange("p (j c) -> p j c", j=2)

            with (
                tc.tile_pool(name="xp", bufs=3) as xp,
                tc.tile_pool(name="ropep", bufs=3) as rp,
                tc.tile_pool(name="exps", bufs=3) as ep,
                tc.tile_pool(name="nrm", bufs=3) as rnp,
                tc.tile_pool(name="stgp", bufs=4) as sgp,
                tc.tile_pool(name="ps", bufs=1, space="PSUM") as ps,
            ):
                # ---------------- QKV units ----------------
                def qk_unit(tci, xt_, wsb, dst, p):
                    tsl = slice(512 * tci, 512 * tci + 512)
                    pq = ps.tile([128, 512], F32, name="pq", tag="pq", bufs=2)
                    for db in range(ND):
                        nc.tensor.matmul(
                            pq[:],
                            lhsT=wsb[:, 256 * db + 128 * p:
                                     256 * db + 128 * p + 128],
                            rhs=xt_[:, 512 * db:512 * db + 512],
                            start=(db == 0), stop=(db == ND - 1),
                        )
                    # RoPE: dst = pq*cos + swap(pq)*sin_signed.  sin_sb is
                    # pre-swapped host-side so the partition swap happens on
                    # the product, via 4 SBUF->SBUF DMAs on the Pool queue.
                    prod = rp.tile([128, 512], MM, name="prod", tag="prod")
                    nc.vector.tensor_tensor(prod[:], pq[:], sin_sb[:, tsl], OP.mult)
                    swp = rp.tile([128, 512], MM, name="swp", tag="swp")
                    for h2 in range(4):
                        b0 = 32 * h2
                        s0 = b0 + (32 if h2 % 2 == 0 else -32)
                        nc.gpsimd.dma_start(out=swp[b0:b0 + 32, :],
                                            in_=prod[s0:s0 + 32, :])
                    nc.vector.tensor_tensor(dst[p][:, tsl], pq[:], cos_sb[:, tsl], OP.mult)
                    # SBUF-only add goes to the Pool engine (PSUM is
                    # GPSIMD-inaccessible on HW, but this one is pure SBUF)
                    nc.gpsimd.tensor_tensor(dst[p][:, tsl], dst[p][:, tsl], swp[:], OP.add)

                def v_unit(tci, tb4, xt_):
                    tb = 4 * tci + tb4
                    pv = ps.tile([128, O_PC], F32, name="pv", tag="pq", bufs=2)
                    for db in range(ND):
                        nc.tensor.matmul(
                            pv[:],
                            lhsT=xt_[:, 512 * db + 128 * tb4:
                                     512 * db + 128 * tb4 + 128],
                            rhs=wv_sb[:, 256 * db:256 * db + 256],
                            start=(db == 0), stop=(db == ND - 1),
                        )
                    for p in range(2):
                        dstv = vsb[p].rearrange(
                            "p (t j c) -> p t j c", j=2, c=65)[:, tb, :, 0:64]
                        srcv = pv[:, 128 * p:128 * p + 128].rearrange(
                            "p (j c) -> p j c", j=2)
                        nc.vector.tensor_copy(dstv, srcv)

                # ------------- unit scheduling machinery -------------
                # uq: qkv units not yet emitted, FIFO with keys so windows
                # can force-emit what they depend on.  mq: deferred
                # transposes + projection blocks (freshness: consumed a
                # beat after they are pushed).
                xts = [xt0, xt1, xt2, xt3]

                uq: list = []   # (key, fn)
                for tci in range(4):
                    uq.append((f"q{tci}p0", lambda t=tci: qk_unit(t, xts[t], wq_sb, qt, 0)))
                    uq.append((f"k{tci}p0", lambda t=tci: qk_unit(t, xts[t], wk_sb, kt, 0)))
                    for tb4 in range(4):
                        uq.append((f"v{tci}.{tb4}",
                                   lambda t=tci, b=tb4: v_unit(t, b, xts[t])))
                    uq.append((f"q{tci}p1", lambda t=tci: qk_unit(t, xts[t], wq_sb, qt, 1)))
                    uq.append((f"k{tci}p1", lambda t=tci: qk_unit(t, xts[t], wk_sb, kt, 1)))
                emitted: set = set()
                kbc = [0]

                def pop_unit(ration=False):
                    if not uq:
                        return False
                    # hold chunk-3 units back as late-phase PE filler: the
                    # big attention windows run at the ScalarE exp cadence
                    # and need ~400ns/kb of extra PE work to stay busy
                    if ration and uq[0][0][1] == "3" and kbc[0] % 4 != 0:
                        return False
                    key, fn = uq.pop(0)
                    fn()
                    emitted.add(key)
                    return True

                def need(key):
                    while key not in emitted and uq:
                        pop_unit()

                mq: list = []   # (pushed_at_kbc, fn)

                def push_misc(fn):
                    mq.append((kbc[0], fn))

                def pop_misc(n=1, force=False):
                    for _ in range(n):
                        # freshness: an item pushed this kb depends on a
                        # DVE chain that has not drained yet - popping it
                        # now would stall the PE on it
                        if mq and (force or mq[0][0] < kbc[0]):
                            mq.pop(0)[1]()
                        else:
                            break

                # ------------- projection -------------
                tp_done = [[False] * NT for _ in range(2)]

                def emit_proj(tb, dc):
                    psy = ps.tile([128, 512], F32, name="psy", tag="pq", bufs=2)
                    for p in range(2):
                        nc.tensor.matmul(
                            psy[:],
                            lhsT=ot[p][:, 128 * tb:128 * tb + 128],
                            rhs=wo_sb[:, 1024 * p + 512 * dc:
                                      1024 * p + 512 * dc + 512],
                            start=(p == 0), stop=(p == 1),
                        )
                    stg = sgp.tile([128, 512], MM, name="stg", tag="stg")
                    nc.vector.tensor_copy(stg[:], psy[:])
                    nc.sync.dma_start(
                        out=y[128 * tb:128 * tb + 128, 512 * dc:512 * dc + 512],
                        in_=stg[:])

                def emit_transpose(p, tb, o_nat):
                    tp = ps.tile([128, 128], MM, name="tp", tag="pq", bufs=2)
                    nc.tensor.transpose(tp[:], o_nat[:], idn_sb[:])
                    nc.vector.tensor_copy(ot[p][:, 128 * tb:128 * tb + 128], tp[:])
                    tp_done[p][tb] = True
                    if tp_done[0][tb] and tp_done[1][tb]:
                        push_misc(lambda t=tb: emit_proj(t, 0))
                        push_misc(lambda t=tb: emit_proj(t, 1))

                # ------------- attention window -------------
                def attn_window(p, w, last=False):
                    pvt = [ps.tile([128, 260], F32, name=f"pv{h}",
                                   tag=f"pv{h}", bufs=1) for h in range(2)]
                    nc.vector.memset(pvt[0][:], 0.0)
                    nc.vector.memset(pvt[1][:], 0.0)

                    def emit_pv(es, kb, r):
                        # diagonal q-block last: its PV also waits on the
                        # mask TensorTensor, the others only on the exp
                        order = list(range(max(r, 0), 4))
                        if r >= 0:
                            order = order[1:] + order[:1]
                        for qbl in order:
                            for i in range(2):
                                nc.tensor.matmul(
                                    pvt[qbl // 2][:, 130 * (qbl % 2) + 65 * i:
                                                  130 * (qbl % 2) + 65 * i + 65],
                                    lhsT=es[:, 512 * i + 128 * qbl:
                                            512 * i + 128 * qbl + 128],
                                    rhs=vsb[p][:, 130 * kb + 65 * i:
                                               130 * kb + 65 * i + 65],
                                    start=False, stop=(r == qbl),
                                    skip_group_check=True,
                                )
                        if r >= 0:
                            # q-block r just finished: normalize now,
                            # transpose on a later beat
                            pvq = pvt[r // 2][:, 130 * (r % 2):
                                              130 * (r % 2) + 130]
                            rec = rnp.tile([128, 2], F32, name="rec", tag="rec")
                            pv3 = pvq.rearrange("p (j c) -> p j c", j=2, c=65)
                            nc.vector.reciprocal(rec[:], pv3[:, :, 64])
                            o_nat = rnp.tile([128, 128], MM, name="o_nat",
                                             tag="o_nat")
                            for i in range(2):
                                nc.vector.tensor_scalar_mul(
                                    o_nat[:, 64 * i:64 * i + 64],
                                    pv3[:, i, 0:64], rec[:, i:i + 1])
                            tb = 4 * w + r
                            push_misc(
                                lambda pp_=p, tb_=tb, o_=o_nat:
                                emit_transpose(pp_, tb_, o_))

                    prev = None   # (es, kb, r) PV runs one kb behind scores
                    for kb in range(4 * w + 4):
                        r = kb - 4 * w
                        c0 = 128 * r if r > 0 else 0
                        need(f"k{kb // 4}p{p}")
                        need(f"v{kb // 4}.{kb % 4}")
                        sc = ps.tile([128, 1024], F32, name="sc", tag="sc",
                                     bufs=2)
                        for i in range(2):
                            nc.tensor.matmul(
                                sc[:, 512 * i + c0:512 * i + 512],
                                lhsT=kt[p][64 * i:64 * i + 64,
                                           128 * kb:128 * kb + 128],
                                rhs=qt[p][64 * i:64 * i + 64,
                                          512 * w + c0:512 * w + 512],
                                start=True, stop=True,
                            )
                        es = ep.tile([128, 1024], MM, name="es", tag="es")
                        sc3 = sc.rearrange("p (j c) -> p j c", j=2)
                        es3 = es.rearrange("p (j c) -> p j c", j=2)
                        nc.scalar.activation(es3[:, :, c0:], sc3[:, :, c0:],
                                             AF.Exp)
                        if r >= 0:
                            # masked diag block; Pool is idle in the late
                            # windows and this TT is SBUF-only
                            nc.gpsimd.tensor_tensor(
                                es3[:, :, c0:c0 + 128],
                                es3[:, :, c0:c0 + 128], msk3, OP.mult)
                        if prev is not None:
                            emit_pv(*prev)
                        # fill the PE while ScalarE streams the exp
                        kbc[0] += 1
                        popped = pop_unit(ration=True)
                        pop_misc(1 if popped else (3 if last else 2),
                                 force=last)
                        prev = (es, kb, r)
                    emit_pv(*prev)

                # ------------- emission stream -------------
                need("q0p0")
                need("k0p0")
                need("v0.0")
                for w in range(NW):
                    for p in range(2):
                        need(f"q{w}p{p}")
                        # prefetch the next window's q/k so their RoPE
                        # chains (DVE/Pool) finish before the scores need them
                        if p == 0:
                            need(f"q{w}p1")
                        elif w + 1 < NW:
                            need(f"q{w + 1}p0")
                        attn_window(p, w, last=(w == NW - 1))
                while uq:
                    pop_unit()
                while mq:
                    pop_misc(force=True)
    nc.compile()
    return nc


def _get_nc():
    if "nc" not in _CACHE:
        _CACHE["nc"] = _build_nc()
    return _CACHE["nc"]


def _np_mm_dtype():
    if MM_DT_NAME == "float16":
        return np.float16
    import ml_dtypes
    return ml_dtypes.bfloat16


def _make_in_maps(x, Wqkv, Wout):
    cosT, sinT, mst, idn = _host_consts()
    ndt = _np_mm_dtype()
    x = np.asarray(x, dtype=np.float32)
    Wqkv = np.asarray(Wqkv, dtype=np.float32)
    Wout = np.asarray(Wout, dtype=np.float32)
    in_maps = []
    for c in range(N_CORES):
        b, g = divmod(c, GROUPS)
        o0 = O_PC * g
        wq = (Wqkv[o0:o0 + O_PC] * np.float32(HD ** -0.5)).astype(np.float32)
        wk = Wqkv[D + o0:D + o0 + O_PC]
        wv = Wqkv[2 * D + o0:2 * D + o0 + O_PC]
        def pack_w(wT):
            # wT: [D, O_PC] -> [128, ND*O_PC] with d-blocks side by side
            return np.ascontiguousarray(
                wT.reshape(ND, 128, O_PC).transpose(1, 0, 2).reshape(128, -1)
            ).astype(ndt)

        xt = x[b].T  # [D, T]
        xtr = np.ascontiguousarray(
            xt.reshape(ND, 128, 4, 512).transpose(2, 1, 0, 3).reshape(4, 128, -1)
        ).astype(ndt)
        wot = Wout[:, o0:o0 + O_PC].T  # [O_PC, D]
        wor = np.ascontiguousarray(
            wot.reshape(2, 128, D).transpose(1, 0, 2).reshape(128, -1)
        ).astype(ndt)
        in_maps.append({
            "xTr": xtr,
            "wqr": pack_w(wq.T), "wkr": pack_w(wk.T), "wvr": pack_w(wv.T),
            "wor": wor,
            "cosT": cosT.astype(ndt), "sinT": sinT.astype(ndt),
            "mst": mst.astype(ndt), "idn": idn.astype(ndt),
        })
    return in_maps


def run(x, Wqkv, Wout, trace=False, **spmd_kwargs):
    from concourse.bass_utils import run_bass_kernel_spmd
    nc = _get_nc()
    in_maps = _make_in_maps(x, Wqkv, Wout)
    res = run_bass_kernel_spmd(nc, in_maps, core_ids=list(range(N_CORES)),
                               trace=trace, **spmd_kwargs)
    out = np.zeros((B, T, D), np.float32)
    for c in range(N_CORES):
        out[c // GROUPS] += res.results[c]["y"].astype(np.float32)
    return out, res


def kernel(x, Wqkv, Wout, mask=None, **_ignored):
    out, _ = run(x, Wqkv, Wout, trace=False)
    return out
